# revision 1
# baseline (speedup 1.0000x reference)
"""GNN message-passing NodeBlock kernel for 8 Trainium2 NeuronCores.

Problem:
    agg_a = segment_sum(edata_a, conn_a[1], 100000)   # [N, 64]
    agg_b = segment_sum(edata_b, conn_b[1], 100000)   # [N, 64]
    out   = concat([agg_a, agg_b, vdata], 1) @ W + b  # [N, 128]

Sharding strategy (chosen; replaces the all-reduce suggestion):
    Edges are sharded BY RECEIVER RANGE — core c owns nodes
    [c*12544, (c+1)*12544) and receives exactly the edges targeting them, so
    each core computes its slice of the aggregation completely locally and no
    collective is needed. Within a core, edges are binned into 128-node
    windows; each 128-edge tile is scattered into its window via a one-hot
    selection matrix (built on DVE with is_equal against an iota row) and a
    PE matmul accumulated in PSUM. Edge features travel as an exact bf16
    hi/lo split (hi = bf16(x), lo = bf16(x - hi)) so the scatter matmul runs
    at full bf16 PE rate while keeping ~2^-18 relative accuracy; the hi and
    lo column blocks are folded after each window. The dense updater runs as
    fp32 matmuls over 512-node column blocks on the transposed layout
    (out^T = W^T x^T), interleaved with phase 1.

SPMD: one program for all 8 cores. Per-(core,window) tile counts differ, so
windows are sorted by edge count per core and the per-step tile count is the
max across cores (order statistics align, so padding stays small). Padding
slots carry rel=-1 (matches no iota column) and zero data.
"""
import numpy as np
import ml_dtypes

import concourse.bass as bass
import concourse.tile as tile
from concourse import mybir
from concourse.bass_utils import run_bass_kernel_spmd
from concourse.vector_clock import ScopedClock

BF16 = ml_dtypes.bfloat16

N_NODES = 100000
N_EDGES = 800000
D_EDGE = 64
D_NODE = 128
D_OUT = 128
N_CORES = 8
W = 128                    # nodes per window
WPC = 98                   # windows per core
NPC = W * WPC              # nodes per core (12544)
NTOT = NPC * N_CORES       # padded node space (100352)
BLK_STEPS = 4              # windows per phase-2 block (4*128 = 512 cols)
N_BLKS = (WPC + BLK_STEPS - 1) // BLK_STEPS  # 25 (last block has 2 steps)

# ---------------------------------------------------------------------------
# compat patches for this container's walrus build
# ---------------------------------------------------------------------------

_MAX_WAITS = 1


def _patched_drain_and_barrier(self, tick_clock, wait_clock):
    nc = self.nc
    probe = nc.sync.nop(nofuse=True, hint="tile_drain_wait0")
    wait_clock.add_sem_waits(
        probe.ins, ScopedClock({None: tick_clock.global_clock})
    )
    si = probe.ins.sync_info
    waits = list(si.on_wait) if si is not None and si.on_wait else []
    if len(waits) > _MAX_WAITS:
        si.on_wait = waits[:_MAX_WAITS]
        for k in range(_MAX_WAITS, len(waits), _MAX_WAITS):
            n = nc.sync.nop(nofuse=True, hint=f"tile_drain_wait{k}")
            n.ins.sync_info = mybir.SyncInfo(
                on_wait=waits[k : k + _MAX_WAITS], on_update=[]
            )
    drain_inst = nc.sync.drain()
    wait_clock.add_sem_waits(
        drain_inst.ins, ScopedClock({None: tick_clock.global_clock})
    )
    dsi = drain_inst.ins.sync_info
    if dsi is not None and dsi.on_wait and len(dsi.on_wait) > _MAX_WAITS:
        dsi.on_wait = []
    nc.all_engine_barrier()
    assert self.sems is not None
    popped = nc._tile_sem_poison_stack.pop()
    assert popped is self._sem_poison
    nc.clear_and_free_semaphores(list(self.sems.allocated().values()))
    nc.all_engine_barrier()


def _split_multi_waits(nc):
    """This walrus build accepts one sync-wait per TPB instruction; move
    extra waits onto preceding same-engine NOPs."""
    for fn in nc.m.functions:
        for blk in fn.blocks:
            out = []
            changed = False
            for inst in blk.instructions:
                si = inst.sync_info
                if si is not None and si.on_wait and len(si.on_wait) > 1:
                    waits = list(si.on_wait)
                    for j, w in enumerate(waits[:-1]):
                        nop = mybir.InstNoOp(
                            name=f"{inst.name}_xw{j}", ins=[], outs=[]
                        )
                        nop.engine = inst.engine
                        nop.sync_info = mybir.SyncInfo(
                            on_wait=[w], on_update=[]
                        )
                        out.append(nop)
                    si.on_wait = [waits[-1]]
                    changed = True
                out.append(inst)
            if changed:
                blk.instructions = out


def _install_ntff_hook_shim():
    import sys
    import types

    if "antenv.axon_hooks" in sys.modules:
        return
    mod = types.ModuleType("antenv.axon_hooks")
    _hook = [None]
    mod.set_axon_ntff_profile_hook = lambda h: _hook.__setitem__(0, h)
    mod.get_axon_ntff_profile_hook = lambda: _hook[0]
    sys.modules["antenv.axon_hooks"] = mod
    try:
        import antenv

        antenv.axon_hooks = mod
    except ImportError:
        pass
    try:
        from trn_agent_boot.trn_boot import _ntff_profile_via_ctypes

        mod.set_axon_ntff_profile_hook(
            _ntff_profile_via_ctypes("/opt/axon/libaxon_pjrt.so")
        )
    except Exception:
        pass


tile.TileContext._drain_and_barrier = _patched_drain_and_barrier
_install_ntff_hook_shim()

# ---------------------------------------------------------------------------
# host-side sharding / packing
# ---------------------------------------------------------------------------


def _pack_type(edata, recv, perms):
    """Bin edges by receiver window, order windows per core by perms, and
    compute the shared per-step tile counts (max across cores).

    Returns (counts[c][w_sorted], sorted edge ids grouped per core+window).
    """
    gwin = recv >> 7  # global window id, 0..783 (core = gwin // 98)
    order = np.argsort(gwin, kind="stable")
    sorted_ids = order
    counts = np.bincount(gwin, minlength=WPC * N_CORES)
    starts = np.zeros(WPC * N_CORES + 1, dtype=np.int64)
    np.cumsum(counts, out=starts[1:])
    return sorted_ids, counts, starts


def _preprocess(vdata, edata_a, edata_b, conn_a, conn_b, W_mat, b_vec):
    recv_a = np.asarray(conn_a[1]).astype(np.int64)
    recv_b = np.asarray(conn_b[1]).astype(np.int64)
    ids_a, cnt_a, st_a = _pack_type(edata_a, recv_a, None)
    ids_b, cnt_b, st_b = _pack_type(edata_b, recv_b, None)

    cnt_a2 = cnt_a.reshape(N_CORES, WPC)
    cnt_b2 = cnt_b.reshape(N_CORES, WPC)

    # per-core window order: heaviest combined windows first
    perms = np.argsort(-(cnt_a2 + cnt_b2), axis=1, kind="stable")  # [C, WPC]

    ta = -np.sort(-np.ceil(cnt_a2 / 128).astype(np.int32), axis=1)
    tb = -np.sort(-np.ceil(cnt_b2 / 128).astype(np.int32), axis=1)
    # tile counts at each step, in each core's own sorted order
    tiles_a = np.ceil(np.take_along_axis(cnt_a2, perms, 1) / 128).astype(np.int32)
    tiles_b = np.ceil(np.take_along_axis(cnt_b2, perms, 1) / 128).astype(np.int32)
    na_step = np.maximum(tiles_a.max(axis=0), 1)  # [WPC]
    nb_step = np.maximum(tiles_b.max(axis=0), 1)
    del ta, tb

    Ta = int(na_step.sum())
    Tb = int(nb_step.sum())
    offs_a = np.zeros(WPC + 1, np.int64)
    np.cumsum(na_step, out=offs_a[1:])
    offs_b = np.zeros(WPC + 1, np.int64)
    np.cumsum(nb_step, out=offs_b[1:])

    # exact bf16 hi/lo split of the edge features
    def hilo(e):
        hi = e.astype(BF16)
        lo = (e - hi.astype(np.float32)).astype(BF16)
        return np.concatenate([hi, lo], axis=1)  # [E, 128] bf16

    eh_a_full = hilo(np.asarray(edata_a))
    eh_b_full = hilo(np.asarray(edata_b))

    def pack_core(c, ids, starts, cnts2, perm, n_step, offs, T, eh_full, recv):
        # slot -> edge id (or -1)
        slot_eid = np.full(T * 128, -1, dtype=np.int64)
        slot_rel = np.full(T * 128, -1.0, dtype=np.float32)
        for i in range(WPC):
            w = perm[i]
            g = c * WPC + w
            cnt = cnts2[c, w]
            if cnt == 0:
                continue
            eids = ids[starts[g] : starts[g] + cnt]
            s0 = offs[i] * 128
            slot_eid[s0 : s0 + cnt] = eids
            slot_rel[s0 : s0 + cnt] = (recv[eids] & 127).astype(np.float32)
        gath = eh_full[np.maximum(slot_eid, 0)]
        gath[slot_eid < 0] = 0
        eh = np.ascontiguousarray(
            gath.reshape(T, 128, 128).transpose(1, 0, 2)
        )  # [slot, tile, feat]
        rel = np.ascontiguousarray(slot_rel.reshape(T, 128).T)  # [128, T]
        return eh, rel

    vdata = np.asarray(vdata)
    vpad = np.zeros((NTOT, D_NODE), dtype=np.float32)
    vpad[:N_NODES] = vdata

    in_maps = []
    iota = np.ascontiguousarray(
        np.broadcast_to(np.arange(128, dtype=np.float32), (128, 128))
    ).astype(BF16)
    Wf = np.ascontiguousarray(np.asarray(W_mat), dtype=np.float32)
    bf = np.asarray(b_vec).astype(np.float32).reshape(D_OUT, 1)

    for c in range(N_CORES):
        eh_a, rel_a = pack_core(
            c, ids_a, st_a, cnt_a2, perms[c], na_step, offs_a, Ta, eh_a_full, recv_a
        )
        eh_b, rel_b = pack_core(
            c, ids_b, st_b, cnt_b2, perms[c], nb_step, offs_b, Tb, eh_b_full, recv_b
        )
        # node columns in permuted window order
        base = c * NPC
        nodes = (
            base
            + (perms[c][:, None] * 128 + np.arange(128)[None, :]).reshape(-1)
        )
        vT = np.ascontiguousarray(vpad[nodes].T)  # [128, NPC]
        in_maps.append(
            {
                "eh_a": eh_a,
                "rel_a": rel_a,
                "eh_b": eh_b,
                "rel_b": rel_b,
                "vT": vT,
                "Wd": Wf,
                "bd": bf,
                "iota": iota,
            }
        )

    sched = (tuple(int(x) for x in na_step), tuple(int(x) for x in nb_step))
    return in_maps, sched, perms


# ---------------------------------------------------------------------------
# device kernel
# ---------------------------------------------------------------------------

_NC_CACHE = {}


def _build(sched):
    na_step, nb_step = sched
    Ta = sum(na_step)
    Tb = sum(nb_step)
    max_na = max(na_step)
    max_nb = max(nb_step)
    f32 = mybir.dt.float32
    bf16 = mybir.dt.bfloat16

    nc = bass.Bass(trn_type="TRN2")
    eh_a_d = nc.dram_tensor("eh_a", [128, Ta, 128], bf16, kind="ExternalInput")
    rel_a_d = nc.dram_tensor("rel_a", [128, Ta], f32, kind="ExternalInput")
    eh_b_d = nc.dram_tensor("eh_b", [128, Tb, 128], bf16, kind="ExternalInput")
    rel_b_d = nc.dram_tensor("rel_b", [128, Tb], f32, kind="ExternalInput")
    vT_d = nc.dram_tensor("vT", [128, NPC], f32, kind="ExternalInput")
    W_d = nc.dram_tensor("Wd", [2 * D_NODE, D_OUT], f32, kind="ExternalInput")
    b_d = nc.dram_tensor("bd", [D_OUT, 1], f32, kind="ExternalInput")
    iota_d = nc.dram_tensor("iota", [128, 128], bf16, kind="ExternalInput")
    outT_d = nc.dram_tensor("outT", [128, NPC], f32, kind="ExternalOutput")

    with tile.TileContext(nc) as tc:
        with (
            tc.tile_pool(name="consts", bufs=1) as cb,
            tc.tile_pool(name="vx", bufs=N_BLKS) as vxp,
            tc.tile_pool(name="x0", bufs=N_BLKS) as x0p,
            tc.tile_pool(name="edges", bufs=3) as ep,
            tc.tile_pool(name="sel", bufs=6) as sp,
            tc.tile_pool(name="out", bufs=3) as op,
            tc.tile_pool(name="psum1", bufs=4, space="PSUM") as pp1,
            tc.tile_pool(name="psum2", bufs=2, space="PSUM") as pp2,
        ):
            iota_sb = cb.tile([128, 128], bf16)
            nc.sync.dma_start(iota_sb[:], iota_d[:, :])
            w0_sb = cb.tile([128, D_OUT], f32, tag="w0")
            nc.sync.dma_start(w0_sb[:], W_d[0:128, :])
            w1_sb = cb.tile([128, D_OUT], f32, tag="w1")
            nc.sync.dma_start(w1_sb[:], W_d[128:256, :])
            b_sb = cb.tile([D_OUT, 1], f32, tag="b")
            nc.sync.dma_start(b_sb[:], b_d[:, :])
            rel_a_sb = cb.tile([128, Ta], f32, tag="rel_a")
            nc.sync.dma_start(rel_a_sb[:], rel_a_d[:, :])
            rel_b_sb = cb.tile([128, Tb], f32, tag="rel_b")
            nc.sync.dma_start(rel_b_sb[:], rel_b_d[:, :])

            vts = []
            for j in range(N_BLKS):
                c0 = j * BLK_STEPS * W
                cols = min(BLK_STEPS * W, NPC - c0)
                vt = vxp.tile([128, cols], f32, tag="vt")
                nc.sync.dma_start(vt[:], vT_d[:, c0 : c0 + cols])
                vts.append(vt)

            offa = 0
            offb = 0
            for j in range(N_BLKS):
                i0 = j * BLK_STEPS
                steps = min(BLK_STEPS, WPC - i0)
                cols_blk = steps * W
                x0 = x0p.tile([128, cols_blk], f32, tag="x0")
                for s in range(steps):
                    i = i0 + s
                    col = s * W
                    for half, (eh_d, rel_sb, n_t, off) in enumerate(
                        (
                            (eh_a_d, rel_a_sb, na_step[i], offa),
                            (eh_b_d, rel_b_sb, nb_step[i], offb),
                        )
                    ):
                        et = ep.tile([128, max(max_na, max_nb) * 128], bf16,
                                     tag="et")
                        nc.sync.dma_start(
                            et[:, : n_t * 128], eh_d[:, off : off + n_t, :]
                        )
                        ps = pp1.tile([128, W], f32, tag="p1")
                        for k in range(n_t):
                            sel = sp.tile([128, W], bf16, tag="sel")
                            nc.vector.tensor_scalar(
                                out=sel[:],
                                in0=iota_sb[:],
                                scalar1=rel_sb[:, off + k : off + k + 1],
                                scalar2=None,
                                op0=mybir.AluOpType.is_equal,
                            )
                            nc.tensor.matmul(
                                out=ps[:],
                                lhsT=et[:, k * 128 : (k + 1) * 128],
                                rhs=sel[:],
                                start=(k == 0),
                                stop=(k == n_t - 1),
                            )
                        r0 = half * 64
                        nc.scalar.copy(
                            x0[r0 : r0 + 64, col : col + W], ps[0:64, :]
                        )
                        nc.vector.tensor_add(
                            x0[r0 : r0 + 64, col : col + W],
                            x0[r0 : r0 + 64, col : col + W],
                            ps[64:128, :],
                        )
                    offa += na_step[i]
                    offb += nb_step[i]

                po = pp2.tile([128, cols_blk], f32, tag="p2")
                nc.tensor.matmul(
                    out=po[:], lhsT=w0_sb[:], rhs=x0[:], start=True, stop=False
                )
                nc.tensor.matmul(
                    out=po[:], lhsT=w1_sb[:], rhs=vts[j][:],
                    start=False, stop=True,
                )
                ot = op.tile([128, cols_blk], f32, tag="ot")
                nc.scalar.activation(
                    out=ot[:],
                    in_=po[:],
                    func=mybir.ActivationFunctionType.Identity,
                    bias=b_sb[:, 0:1],
                    scale=1.0,
                )
                nc.sync.dma_start(
                    outT_d[:, i0 * W : i0 * W + cols_blk], ot[:]
                )
    _split_multi_waits(nc)
    return nc


# ---------------------------------------------------------------------------
# public entry point
# ---------------------------------------------------------------------------


def kernel(vdata, edata_a, edata_b, conn_a, conn_b, W, b, _trace=False):
    in_maps, sched, perms = _preprocess(
        vdata, edata_a, edata_b, conn_a, conn_b, W, b
    )
    nc = _NC_CACHE.get(sched)
    if nc is None:
        nc = _build(sched)
        _NC_CACHE[sched] = nc
    kwargs = {}
    if _trace:
        kwargs = dict(trace=True, trace_cores=[0])
    res = run_bass_kernel_spmd(
        nc, in_maps, core_ids=list(range(N_CORES)), **kwargs
    )

    out_full = np.empty((NTOT, D_OUT), dtype=np.float32)
    for c in range(N_CORES):
        outT = res.results[c]["outT"]  # [128, NPC]
        blocks = outT.reshape(D_OUT, WPC, 128)
        base = c * NPC
        for i in range(WPC):
            w = perms[c][i]
            out_full[base + w * 128 : base + (w + 1) * 128] = blocks[:, i, :].T
    out = out_full[:N_NODES]
    if _trace:
        return out, res
    return out
